# revision 8
# baseline (speedup 1.0000x reference)
"""KoLeo loss (distributed) on 8 Trainium2 NeuronCores.

Strategy: data-parallel over rows. Host normalizes x (the cheap part,
0.05% of FLOPs) and stages the normalized embeddings transposed +
replicated to every core (this is the all-gather, done at input staging).
Each core computes its [1024, 8192] slice of the Gram matrix with a
resident-SBUF bf16 GEMM and extracts the per-row top-8 dot products with
the DVE max instruction directly from PSUM. Because rows are unit-norm,
the self-dot (=1) always ranks first, so no diagonal masking is needed,
and nearest-neighbor distances follow from d^2 = 2 - 2*dot without any
gather. Host reduces the 8x[1024,8] top-8 tables to the scalar loss in
float64.
"""

import sys

sys.path.insert(0, "/opt/trn_rl_repo")

import numpy as np
import ml_dtypes

import concourse.bass as bass
import concourse.tile as tile
from concourse import mybir
from concourse.bass import ds, ts
from concourse.vector_clock import ScopedClock
from concourse.bass_utils import run_bass_kernel_spmd

B = 8192
D = 1024
NCORES = 8
P = 128
MT = (B // NCORES) // P  # 8 row-tiles per core
KC = D // P  # 8 contraction chunks
NW = 4  # column windows of 4 psum banks
WJ = 4  # 512-wide chunks per window
WIN = WJ * 512  # 2048 columns per window

TOPK = 2
GATE_THRESHOLD = 0.5
GATE_ALPHA = 0.1
EPS = 1e-8


class PatchedTileContext(tile.TileContext):
    """The tail drain in this walrus build only tolerates a single sem wait
    per instruction; spill the rest onto standalone wait instructions."""

    def _drain_and_barrier(self, tick_clock, wait_clock):
        nc = self.nc
        drain_inst = nc.sync.drain()
        wait_clock.add_sem_waits(
            drain_inst.ins, ScopedClock({None: tick_clock.global_clock})
        )
        si = drain_inst.ins.sync_info
        if si is not None and len(si.on_wait) > 1:
            waits = list(si.on_wait)
            si.on_wait = waits[:1]
            id2sem = {h.num: h for h in self.sems.allocated().values()}
            for w in waits[1:]:
                nc.sync.wait_ge(id2sem[w.id], w.wait_value)
        nc.all_engine_barrier()
        popped = nc._tile_sem_poison_stack.pop()
        assert popped is self._sem_poison
        nc.clear_and_free_semaphores(list(self.sems.allocated().values()))
        nc.all_engine_barrier()


def _split_excess_waits(nc, max_waits=1):
    """This walrus build rejects instructions carrying more than one sem
    wait; hoist extras onto standalone EventSemaphore instructions placed
    immediately before the over-subscribed instruction on the same engine
    (engines dispatch in order, so this is semantically identical)."""
    for fn in nc.m.functions:
        for bb in fn.blocks:
            insts = bb.instructions
            out = []
            for inst in insts:
                si = inst.sync_info
                if si is not None and len(si.on_wait) > max_waits:
                    waits = list(si.on_wait)
                    for w in waits[:-max_waits]:
                        ev = mybir.InstEventSemaphore(
                            name=nc.get_next_instruction_name(), ins=[], outs=[]
                        )
                        ev.engine = inst.engine
                        ev.sync_info = mybir.SyncInfo(on_wait=[w], on_update=[])
                        out.append(ev)
                    si.on_wait = waits[-max_waits:]
                out.append(inst)
            insts[:] = out


def build_program():
    nc = bass.Bass()
    xt_d = nc.declare_dram_parameter(
        "xt", [KC, P, B], mybir.dt.bfloat16, isOutput=False
    )
    lhsT_d = nc.declare_dram_parameter(
        "lhsT", [KC, P, MT * P], mybir.dt.bfloat16, isOutput=False
    )
    out_d = nc.declare_dram_parameter(
        "top8", [MT, P, 8], mybir.dt.float32, isOutput=True
    )

    with PatchedTileContext(nc) as tc:
        with (
            tc.tile_pool(name="xt_pool", bufs=KC * NW) as xt_pool,
            tc.tile_pool(name="w_pool", bufs=KC) as w_pool,
            tc.tile_pool(name="acc_pool", bufs=1) as acc_pool,
            tc.tile_pool(name="psum", bufs=2, space=bass.MemorySpace.PSUM) as psum_pool,
        ):
            # rhs: full xn.T resident, one tile per (k-chunk, column window)
            # so matmuls only depend on the slice they read.
            xt_sb = [
                [
                    xt_pool.tile([P, WIN], mybir.dt.bfloat16, name="xt_rez")
                    for w in range(NW)
                ]
                for k in range(KC)
            ]
            lhsT_sb = [
                w_pool.tile([P, MT * P], mybir.dt.bfloat16, name="lhsT_rez")
                for _ in range(KC)
            ]
            # weights first (every window needs them), then xt in
            # column-window-major order so window 0 arrives first
            for k in range(KC):
                nc.sync.dma_start(lhsT_sb[k][:], lhsT_d[k])
            for w in range(NW):
                for k in range(KC):
                    nc.sync.dma_start(xt_sb[k][w][:], xt_d[k, :, ds(w * WIN, WIN)])

            # per-(m, w) top-8 staging: [p, m, w, 8]
            t8w = acc_pool.tile([P, MT, NW, 8], mybir.dt.float32)
            out_sb = acc_pool.tile([P, MT, 8], mybir.dt.float32)

            for w in range(NW):
                for m in range(MT):
                    psum = psum_pool.tile([P, WJ, 512], mybir.dt.float32)
                    for k in range(KC):
                        lw = lhsT_sb[k][:, ts(m, P)]
                        for j in range(WJ):
                            nc.tensor.matmul(
                                psum[:, j],
                                lw,
                                xt_sb[k][w][:, ts(j, 512)],
                                start=(k == 0),
                                stop=(k == KC - 1),
                            )
                    # top-8 of this 2048-wide window, straight from PSUM
                    nc.vector.max(t8w[:, m, w], psum[:, :, :])

            for m in range(MT):
                nc.vector.max(out_sb[:, m], t8w[:, m, :, :])

            nc.sync.dma_start(out_d.rearrange("mt p v -> p mt v"), out_sb[:])

    _split_excess_waits(nc)
    return nc


_nc_cache = None


def kernel(x: np.ndarray) -> np.ndarray:
    global _nc_cache
    assert x.shape == (B, D)

    # --- host: normalize (fp64), transpose, shard ---
    x64 = x.astype(np.float64)
    norm = np.sqrt(np.sum(x64 * x64, axis=1, keepdims=True))
    xn = x64 / np.maximum(norm, EPS)
    xt = np.ascontiguousarray(xn.T).astype(ml_dtypes.bfloat16)  # [D, B]
    xt_in = xt.reshape(KC, P, B)

    in_maps = []
    for c in range(NCORES):
        lhsT = np.ascontiguousarray(xt_in[:, :, c * MT * P : (c + 1) * MT * P])
        in_maps.append({"xt": xt_in, "lhsT": lhsT})

    if _nc_cache is None:
        _nc_cache = build_program()
    res = run_bass_kernel_spmd(_nc_cache, in_maps, list(range(NCORES)))

    # --- host: reduce top-8 tables to the scalar loss (fp64) ---
    # top8[c][mt, p, v] -> row c*1024 + mt*128 + p
    tops = np.stack([res.results[c]["top8"] for c in range(NCORES)])  # [NC, MT, P, 8]
    v = tops.reshape(B, 8).astype(np.float64)
    # rank 0 is the self-dot (~1.0); ranks 1..TOPK are the nearest neighbors
    vk = v[:, 1 : 1 + TOPK]  # [B, TOPK]
    d2 = np.maximum(2.0 - 2.0 * vk, 0.0)
    distances = np.sqrt(d2).reshape(-1)
    losses = -np.log(distances + EPS)
    alpha = max(GATE_ALPHA, 1e-6)
    gate = 1.0 / (1.0 + np.exp(-(losses - GATE_THRESHOLD) / alpha))
    lg = losses * gate
    weighted_mean = lg.mean()
    gated_mean = lg.sum() / max(gate.sum(), 1.0)
    out = 0.5 * weighted_mean + 0.5 * gated_mean
    return np.float32(out)


# revision 14
# speedup vs baseline: 1.0269x; 1.0269x over previous
"""KoLeo loss (distributed) on 8 Trainium2 NeuronCores.

Strategy: data-parallel over rows. Host normalizes x (the cheap part,
0.05% of FLOPs) and stages the normalized embeddings transposed +
replicated to every core (this is the all-gather, done at input staging).
Each core computes its [1024, 8192] slice of the Gram matrix with a
resident-SBUF bf16 GEMM and extracts the per-row top-8 dot products with
the DVE max instruction directly from PSUM. Because rows are unit-norm,
the self-dot (=1) always ranks first, so no diagonal masking is needed,
and nearest-neighbor distances follow from d^2 = 2 - 2*dot without any
gather. Host reduces the 8x[1024,8] top-8 tables to the scalar loss in
float64.
"""

import sys

sys.path.insert(0, "/opt/trn_rl_repo")

import numpy as np
import ml_dtypes

import concourse.bass as bass
import concourse.tile as tile
from concourse import mybir
from concourse.bass import ds, ts
from concourse.vector_clock import ScopedClock
from concourse.bass_utils import run_bass_kernel_spmd

B = 8192
D = 1024
NCORES = 8
P = 128
MT = (B // NCORES) // P  # 8 row-tiles per core
KC = D // P  # 8 contraction chunks
NW = 4  # column windows of 4 psum banks
WJ = 4  # 512-wide chunks per window
WIN = WJ * 512  # 2048 columns per window

TOPK = 2
GATE_THRESHOLD = 0.5
GATE_ALPHA = 0.1
EPS = 1e-8


class PatchedTileContext(tile.TileContext):
    """The tail drain in this walrus build only tolerates a single sem wait
    per instruction; spill the rest onto standalone wait instructions."""

    def _drain_and_barrier(self, tick_clock, wait_clock):
        nc = self.nc
        drain_inst = nc.sync.drain()
        wait_clock.add_sem_waits(
            drain_inst.ins, ScopedClock({None: tick_clock.global_clock})
        )
        si = drain_inst.ins.sync_info
        if si is not None and len(si.on_wait) > 1:
            waits = list(si.on_wait)
            si.on_wait = waits[:1]
            id2sem = {h.num: h for h in self.sems.allocated().values()}
            for w in waits[1:]:
                nc.sync.wait_ge(id2sem[w.id], w.wait_value)
        nc.all_engine_barrier()
        popped = nc._tile_sem_poison_stack.pop()
        assert popped is self._sem_poison
        nc.clear_and_free_semaphores(list(self.sems.allocated().values()))
        nc.all_engine_barrier()


def _split_excess_waits(nc, max_waits=1):
    """This walrus build rejects instructions carrying more than one sem
    wait; hoist extras onto standalone EventSemaphore instructions placed
    immediately before the over-subscribed instruction on the same engine
    (engines dispatch in order, so this is semantically identical)."""
    for fn in nc.m.functions:
        for bb in fn.blocks:
            insts = bb.instructions
            out = []
            for inst in insts:
                si = inst.sync_info
                if si is not None and len(si.on_wait) > max_waits:
                    waits = list(si.on_wait)
                    for w in waits[:-max_waits]:
                        ev = mybir.InstEventSemaphore(
                            name=nc.get_next_instruction_name(), ins=[], outs=[]
                        )
                        ev.engine = inst.engine
                        ev.sync_info = mybir.SyncInfo(on_wait=[w], on_update=[])
                        out.append(ev)
                    si.on_wait = waits[-max_waits:]
                out.append(inst)
            insts[:] = out


def build_program():
    nc = bass.Bass()
    xt_d = nc.declare_dram_parameter(
        "xt", [KC, P, B], mybir.dt.bfloat16, isOutput=False
    )
    lhsT_d = nc.declare_dram_parameter(
        "lhsT", [KC, P, MT * P], mybir.dt.bfloat16, isOutput=False
    )
    out_d = nc.declare_dram_parameter(
        "top8", [MT, P, 8], mybir.dt.float32, isOutput=True
    )

    with PatchedTileContext(nc) as tc:
        with (
            tc.tile_pool(name="xt_pool", bufs=KC * NW) as xt_pool,
            tc.tile_pool(name="w_pool", bufs=KC) as w_pool,
            tc.tile_pool(name="acc_pool", bufs=1) as acc_pool,
            tc.tile_pool(name="psum", bufs=2, space=bass.MemorySpace.PSUM) as psum_pool,
        ):
            # rhs: full xn.T resident, one tile per (k-chunk, column window)
            # so matmuls only depend on the slice they read.
            xt_sb = [
                [
                    xt_pool.tile([P, WIN], mybir.dt.bfloat16, name="xt_rez")
                    for w in range(NW)
                ]
                for k in range(KC)
            ]
            lhsT_sb = [
                w_pool.tile([P, MT * P], mybir.dt.bfloat16, name="lhsT_rez")
                for _ in range(KC)
            ]
            # interleave weights with window-0 columns so the first matmuls
            # can start as soon as (lhsT_k, xt_k0) pairs land; remaining
            # windows stream in column-major order behind them
            for k in range(KC):
                nc.sync.dma_start(lhsT_sb[k][:], lhsT_d[k])
                nc.sync.dma_start(xt_sb[k][0][:], xt_d[k, :, ds(0, WIN)])
            for w in range(1, NW):
                for k in range(KC):
                    nc.sync.dma_start(xt_sb[k][w][:], xt_d[k, :, ds(w * WIN, WIN)])

            # per-(m, w) top-8 staging: [p, m, w, 8]
            t8w = acc_pool.tile([P, MT, NW, 8], mybir.dt.float32)
            out_sb = acc_pool.tile([P, MT, 8], mybir.dt.float32)

            # warm up the PE HAM clock gate during the DMA prologue so the
            # real matmuls run at full clock from the start
            warm_sb = acc_pool.tile([P, 512], mybir.dt.bfloat16)
            nc.gpsimd.memset(warm_sb[:], 0.0)
            warm_ps = psum_pool.tile([P, WJ, 512], mybir.dt.float32, name="psum")
            for i in range(12):
                nc.tensor.matmul(warm_ps[:, i % WJ], warm_sb[:, :P], warm_sb[:])

            for w in range(NW):
                for m in range(MT):
                    psum = psum_pool.tile([P, WJ, 512], mybir.dt.float32)
                    for k in range(KC):
                        lw = lhsT_sb[k][:, ts(m, P)]
                        for j in range(WJ):
                            nc.tensor.matmul(
                                psum[:, j],
                                lw,
                                xt_sb[k][w][:, ts(j, 512)],
                                start=(k == 0),
                                stop=(k == KC - 1),
                            )
                    # top-8 of this 2048-wide window, straight from PSUM
                    nc.vector.max(t8w[:, m, w], psum[:, :, :])
                    if w == NW - 1:
                        # merge this row-tile's window top-8s and store as
                        # soon as its last window is reduced
                        nc.vector.max(out_sb[:, m], t8w[:, m, :, :])
                        nc.sync.dma_start(out_d[m], out_sb[:, m])

    _split_excess_waits(nc)
    return nc


_nc_cache = None


def kernel(x: np.ndarray) -> np.ndarray:
    global _nc_cache
    assert x.shape == (B, D)

    # --- host: normalize (fp64), transpose, shard ---
    x64 = x.astype(np.float64)
    norm = np.sqrt(np.sum(x64 * x64, axis=1, keepdims=True))
    xn = x64 / np.maximum(norm, EPS)
    xt = np.ascontiguousarray(xn.T).astype(ml_dtypes.bfloat16)  # [D, B]
    xt_in = xt.reshape(KC, P, B)

    in_maps = []
    for c in range(NCORES):
        lhsT = np.ascontiguousarray(xt_in[:, :, c * MT * P : (c + 1) * MT * P])
        in_maps.append({"xt": xt_in, "lhsT": lhsT})

    if _nc_cache is None:
        _nc_cache = build_program()
    res = run_bass_kernel_spmd(_nc_cache, in_maps, list(range(NCORES)))

    # --- host: reduce top-8 tables to the scalar loss (fp64) ---
    # top8[c][mt, p, v] -> row c*1024 + mt*128 + p
    tops = np.stack([res.results[c]["top8"] for c in range(NCORES)])  # [NC, MT, P, 8]
    v = tops.reshape(B, 8).astype(np.float64)
    # rank 0 is the self-dot (~1.0); ranks 1..TOPK are the nearest neighbors
    vk = v[:, 1 : 1 + TOPK]  # [B, TOPK]
    d2 = np.maximum(2.0 - 2.0 * vk, 0.0)
    distances = np.sqrt(d2).reshape(-1)
    losses = -np.log(distances + EPS)
    alpha = max(GATE_ALPHA, 1e-6)
    gate = 1.0 / (1.0 + np.exp(-(losses - GATE_THRESHOLD) / alpha))
    lg = losses * gate
    weighted_mean = lg.mean()
    gated_mean = lg.sum() / max(gate.sum(), 1.0)
    out = 0.5 * weighted_mean + 0.5 * gated_mean
    return np.float32(out)
